# revision 5
# baseline (speedup 1.0000x reference)
"""GAT spatio-temporal model Trainium2 kernel.

Sharding: data-parallel over batch B=8 -> 8 NeuronCores (1 graph each).
Layout: feature-on-partition ("T" tensors are [F, N]); attention computed
in transposed [m, n] layout so softmax denominators come from ones-matmul
column sums and AV products hit PE directly.

Shapes (hardcoded): B=8, N=512, Din=64, H=8, F=128, L=2.
"""
import numpy as np
from contextlib import ExitStack

import concourse.bass as bass
import concourse.tile as tile
from concourse import bacc, mybir
from concourse.bass_utils import run_bass_kernel_spmd
from concourse.masks import make_identity

F32 = mybir.dt.float32
AF = mybir.ActivationFunctionType
OP = mybir.AluOpType

B, N, DIN, H, F, L = 8, 512, 64, 8, 128, 2
NCHUNK = N // 128  # 4
ALPHA = 0.2
LN_EPS = 1e-5

_CACHE = {}


def build_nc():
    nc = bacc.Bacc("TRN2", target_bir_lowering=False, debug=False)

    x_d = nc.dram_tensor("x", [N, DIN], F32, kind="ExternalInput").ap()
    adj_d = nc.dram_tensor("adj", [N, N], mybir.dt.int32, kind="ExternalInput").ap()
    Wp_d = nc.dram_tensor("Wp", [DIN, F], F32, kind="ExternalInput").ap()
    bp_d = nc.dram_tensor("bp", [F], F32, kind="ExternalInput").ap()
    Wh_d = nc.dram_tensor("W_heads", [L, H, F, F], F32, kind="ExternalInput").ap()
    ah_d = nc.dram_tensor("a_heads", [L, H, 2 * F], F32, kind="ExternalInput").ap()
    Wo_d = nc.dram_tensor("W_out", [L, H * F, F], F32, kind="ExternalInput").ap()
    ao_d = nc.dram_tensor("a_out", [L, 2 * F], F32, kind="ExternalInput").ap()
    g_d = nc.dram_tensor("ln_g", [L, F], F32, kind="ExternalInput").ap()
    b_d = nc.dram_tensor("ln_b", [L, F], F32, kind="ExternalInput").ap()
    out_d = nc.dram_tensor("out", [N, F], F32, kind="ExternalOutput").ap()

    with tile.TileContext(nc) as tc, ExitStack() as ctx:
        const = ctx.enter_context(tc.tile_pool(name="const", bufs=1))
        sbig = ctx.enter_context(tc.tile_pool(name="sbig", bufs=3))
        srow = ctx.enter_context(tc.tile_pool(name="srow", bufs=6))
        sexp = ctx.enter_context(tc.tile_pool(name="sexp", bufs=2))
        smulti = ctx.enter_context(tc.tile_pool(name="smulti", bufs=9))
        shd = ctx.enter_context(tc.tile_pool(name="shd", bufs=3))
        smask = ctx.enter_context(tc.tile_pool(name="smask", bufs=4))
        # PSUM: 8 banks total
        pz = ctx.enter_context(tc.tile_pool(name="pz", bufs=2, space="PSUM"))
        pou = ctx.enter_context(tc.tile_pool(name="pou", bufs=2, space="PSUM"))
        pmisc = ctx.enter_context(tc.tile_pool(name="pmisc", bufs=2, space="PSUM"))
        prow = ctx.enter_context(tc.tile_pool(name="prow", bufs=2, space="PSUM"))

        # ---------------- constants ----------------
        ones_row = const.tile([1, N], F32)
        nc.vector.memset(ones_row, 1.0)
        ones_col = const.tile([128, 1], F32)
        nc.vector.memset(ones_col, 1.0)
        ident = const.tile([128, 128], F32)
        make_identity(nc, ident)
        eps1 = const.tile([1, 1], F32)
        nc.vector.memset(eps1, LN_EPS)

        Wp_sb = const.tile([DIN, F], F32)
        nc.sync.dma_start(Wp_sb, Wp_d)
        bp_col = const.tile([F, 1], F32)
        nc.sync.dma_start(bp_col, bp_d.rearrange("(f one) -> f one", one=1))

        Wh_sb = [[const.tile([F, F], F32, name=f"Wh_{l}_{h}") for h in range(H)]
                 for l in range(L)]
        ah_sb = [[const.tile([F, 2], F32, name=f"ah_{l}_{h}") for h in range(H)]
                 for l in range(L)]
        for l in range(L):
            for h in range(H):
                nc.sync.dma_start(Wh_sb[l][h], Wh_d[l, h])
                nc.sync.dma_start(ah_sb[l][h], ah_d[l, h].rearrange("(t f) -> f t", t=2))
        Wo_sb = [const.tile([128, H, F], F32, name=f"Wo_{l}") for l in range(L)]
        ao_sb = [const.tile([F, 2], F32, name=f"ao_{l}") for l in range(L)]
        g_row = [const.tile([1, F], F32, name=f"grow_{l}") for l in range(L)]
        b_row = [const.tile([1, F], F32, name=f"brow_{l}") for l in range(L)]
        g_col = [const.tile([F, 1], F32, name=f"gcol_{l}") for l in range(L)]
        for l in range(L):
            nc.sync.dma_start(Wo_sb[l], Wo_d[l].rearrange("(c p) f -> p c f", p=128))
            nc.sync.dma_start(ao_sb[l], ao_d[l].rearrange("(t f) -> f t", t=2))
            nc.sync.dma_start(g_row[l], g_d[l].rearrange("(one f) -> one f", one=1))
            nc.sync.dma_start(b_row[l], b_d[l].rearrange("(one f) -> one f", one=1))
            nc.sync.dma_start(g_col[l], g_d[l].rearrange("(f one) -> f one", one=1))

        # S_l/S_r persistent pairs for the K=2 outer-sum matmul.
        S_l = [const.tile([2, N], F32, name=f"Sl{i}") for i in range(2)]
        S_r = [const.tile([2, N], F32, name=f"Sr{i}") for i in range(2)]
        for i in range(2):
            nc.vector.memset(S_l[i], 1.0)
            nc.vector.memset(S_r[i], 1.0)

        # ---------------- x -> xT, input projection ----------------
        xT = const.tile([DIN, N], F32)
        for c in range(NCHUNK):
            xc = shd.tile([128, DIN], F32, tag="xchunk")
            nc.sync.dma_start(xc, x_d[bass.ts(c, 128), :])
            pt = pmisc.tile([DIN, 128], F32, tag="pbig")
            nc.tensor.transpose(pt, xc, ident)
            nc.scalar.activation(xT[:, bass.ts(c, 128)], pt, AF.Copy)
        ph = pmisc.tile([128, N], F32, tag="pbig")
        nc.tensor.matmul(ph, Wp_sb, xT, start=True, stop=True)
        hT = sbig.tile([128, N], F32, tag="hT")
        nc.scalar.activation(hT, ph, AF.Relu, bias=bp_col)

        # ---------------- adj -> maskT (fp32, transposed) ----------------
        adj_f = []
        for r in range(NCHUNK):
            ai = shd.tile([128, N], mybir.dt.int32, tag="adji")
            nc.sync.dma_start(ai, adj_d[bass.ts(r, 128), :])
            af = smask.tile([128, N], F32, tag="adjf")
            nc.vector.tensor_copy(af, ai)
            adj_f.append(af)
        maskT = [const.tile([128, N], F32, name=f"maskT{c}") for c in range(NCHUNK)]
        for c in range(NCHUNK):
            pm = pmisc.tile([128, N], F32, tag="pbig")
            for r in range(NCHUNK):
                nc.tensor.transpose(pm[:, bass.ts(r, 128)],
                                    adj_f[r][:, bass.ts(c, 128)], ident)
            nc.scalar.activation(maskT[c], pm, AF.Copy)

        # ---------------- attention block helper ----------------
        def attention(hsrcT, hsrcN, s1p, s2p, hid):
            """Masked softmax attention in transposed layout.

            hsrcT: [F, N] features-on-partition; hsrcN: [128, N] with chunk c at
            [:, c*128:(c+1)*128] holding [m, f]; s1p/s2p: [1,512] psum rows.
            Returns psum_oU [F, N] (unnormalized AV) and r_sb [1, N] recip denom.
            """
            sl, sr = S_l[hid % 2], S_r[hid % 2]
            nc.vector.tensor_copy(sl[0:1, :], s2p)
            s1row = srow.tile([1, N], F32, tag="rowA")
            nc.scalar.activation(s1row, s1p, AF.Copy)
            nc.sync.dma_start(sr[1:2, :], s1row)

            e_all = sexp.tile([128, NCHUNK, N], F32, tag="e_all")
            p_all = sexp.tile([128, NCHUNK, N], F32, tag="p_all")
            for c in range(NCHUNK):
                zp = pz.tile([128, N], F32, tag="z")
                nc.tensor.matmul(zp, sl[:, bass.ts(c, 128)], sr, start=True, stop=True)
                if c == 0:
                    nc.scalar.activation(e_all[:, c, :], zp, AF.Prelu, alpha=ALPHA)
                else:
                    tl = shd.tile([128, N], F32, tag="lrelu_t")
                    nc.vector.tensor_scalar_mul(tl, zp, ALPHA)
                    nc.vector.tensor_tensor(e_all[:, c, :], zp, tl, OP.max)
            nc.scalar.activation(p_all, e_all, AF.Exp)
            for c in range(NCHUNK):
                nc.vector.tensor_tensor(p_all[:, c, :], p_all[:, c, :], maskT[c], OP.mult)
            pden = prow.tile([1, N], F32, tag="prow")
            pou_t = pou.tile([128, N], F32, tag="oU")
            for c in range(NCHUNK):
                nc.tensor.matmul(pden, ones_col, p_all[:, c, :],
                                 start=(c == 0), stop=(c == NCHUNK - 1))
            for c in range(NCHUNK):
                nc.tensor.matmul(pou_t, hsrcN[:, bass.ts(c, 128)], p_all[:, c, :],
                                 start=(c == 0), stop=(c == NCHUNK - 1))
            den_sb = srow.tile([1, N], F32, tag="rowA")
            nc.scalar.activation(den_sb, pden, AF.Copy)
            r_sb = srow.tile([1, N], F32, tag="rowA")
            nc.vector.reciprocal_approx_fast(r_sb, den_sb)
            return pou_t, r_sb

        def normalize(pou_t, r_sb):
            """outT = pou * rep(r). Returns sbuf [F, N]."""
            prep = pmisc.tile([128, N], F32, tag="pbig")
            nc.tensor.matmul(prep, ones_row[:, 0:128], r_sb, start=True, stop=True)
            rep_sb = sbig.tile([128, N], F32, tag="rep")
            nc.scalar.activation(rep_sb, prep, AF.Copy)
            outT = sbig.tile([128, N], F32, tag="outT")
            nc.vector.tensor_tensor(outT, pou_t, rep_sb, OP.mult)
            return outT

        # ---------------- layers ----------------
        for l in range(L):
            residT = hT
            multiT = []
            for h in range(H):
                W = Wh_sb[l][h]
                pT = pmisc.tile([128, N], F32, tag="pbig")
                nc.tensor.matmul(pT, W, hT, start=True, stop=True)
                hprojT = sbig.tile([128, N], F32, tag="hprojT")
                nc.scalar.activation(hprojT, pT, AF.Copy)
                pN = pmisc.tile([128, N], F32, tag="pbig")
                for c in range(NCHUNK):
                    nc.tensor.matmul(pN[:, bass.ts(c, 128)], hT[:, bass.ts(c, 128)],
                                     W, start=True, stop=True)
                hprojN = sbig.tile([128, N], F32, tag="hprojN")
                nc.vector.tensor_copy(hprojN, pN)
                s1p = prow.tile([1, N], F32, tag="prow")
                nc.tensor.matmul(s1p, ah_sb[l][h][:, 0:1], hprojT, start=True, stop=True)
                s2p = prow.tile([1, N], F32, tag="prow")
                nc.tensor.matmul(s2p, ah_sb[l][h][:, 1:2], hprojT, start=True, stop=True)

                pou_t, r_sb = attention(hprojT, hprojN, s1p, s2p, h)
                outT = normalize(pou_t, r_sb)
                # ELU: max(x, min(exp(x),1)-1)
                ex = shd.tile([128, N], F32, tag="elu_ex")
                nc.scalar.activation(ex, outT, AF.Exp)
                nc.vector.tensor_scalar(ex, ex, 1.0, -1.0, OP.min, OP.add)
                mh = smulti.tile([128, N], F32, tag="multi")
                nc.vector.tensor_tensor(mh, outT, ex, OP.max)
                multiT.append(mh)

            # h2T = W_out.T @ multiT
            ph2 = pou.tile([128, N], F32, tag="oU")
            for h in range(H):
                nc.tensor.matmul(ph2, Wo_sb[l][:, h, :], multiT[h],
                                 start=(h == 0), stop=(h == H - 1))
            h2T = sbig.tile([128, N], F32, tag="h2T")
            nc.scalar.activation(h2T, ph2, AF.Copy)
            h2N = sbig.tile([128, N], F32, tag="h2N")
            pn2 = pmisc.tile([128, N], F32, tag="pbig")
            for c in range(NCHUNK):
                nc.tensor.transpose(pn2[:, bass.ts(c, 128)], h2T[:, bass.ts(c, 128)],
                                    ident)
            nc.vector.tensor_copy(h2N, pn2)
            s1p = prow.tile([1, N], F32, tag="prow")
            nc.tensor.matmul(s1p, ao_sb[l][:, 0:1], h2T, start=True, stop=True)
            s2p = prow.tile([1, N], F32, tag="prow")
            nc.tensor.matmul(s2p, ao_sb[l][:, 1:2], h2T, start=True, stop=True)
            pou_t, r_sb = attention(h2T, h2N, s1p, s2p, 0)
            outsT = normalize(pou_t, r_sb)

            # ---- residual + LN over partition dim ----
            xs = sbig.tile([128, N], F32, tag="xs")
            nc.vector.tensor_tensor(xs, outsT, residT, OP.add)
            xsq = sbig.tile([128, N], F32, tag="xsq")
            nc.vector.tensor_tensor(xsq, xs, xs, OP.mult)
            pmu = prow.tile([1, N], F32, tag="prow")
            nc.tensor.matmul(pmu, ones_col, xs, start=True, stop=True)
            psq = prow.tile([1, N], F32, tag="prow")
            nc.tensor.matmul(psq, ones_col, xsq, start=True, stop=True)
            mu = srow.tile([1, N], F32, tag="rowL")
            nc.vector.tensor_scalar_mul(mu, pmu, 1.0 / F)
            msq = srow.tile([1, N], F32, tag="rowL")
            nc.vector.tensor_scalar_mul(msq, psq, 1.0 / F)
            mu2 = srow.tile([1, N], F32, tag="rowL")
            nc.vector.tensor_tensor(mu2, mu, mu, OP.mult)
            var = srow.tile([1, N], F32, tag="rowL")
            nc.vector.tensor_tensor(var, msq, mu2, OP.subtract)
            lnv = srow.tile([1, N], F32, tag="rowL")
            nc.scalar.activation(lnv, var, AF.Ln, bias=eps1)
            rstd = srow.tile([1, N], F32, tag="rowL")
            nc.scalar.activation(rstd, lnv, AF.Exp, scale=-0.5)
            mr = srow.tile([1, N], F32, tag="rowL")
            nc.vector.tensor_tensor(mr, mu, rstd, OP.mult)
            r2 = srow.tile([1, N], F32, tag="rowL")
            nc.vector.tensor_scalar_mul(r2, mr, -1.0)
            paff = pz.tile([128, N], F32, tag="z")
            nc.tensor.matmul(paff, g_row[l], r2, start=True, stop=False)
            nc.tensor.matmul(paff, b_row[l], ones_row, start=False, stop=True)
            prsd = pmisc.tile([128, N], F32, tag="pbig")
            nc.tensor.matmul(prsd, ones_row[:, 0:128], rstd, start=True, stop=True)
            rep_rstd = sbig.tile([128, N], F32, tag="rep")
            nc.scalar.activation(rep_rstd, prsd, AF.Copy)
            y = sbig.tile([128, N], F32, tag="y")
            nc.vector.tensor_tensor(y, xs, rep_rstd, OP.mult)
            nc.vector.tensor_scalar_mul(y, y, g_col[l])
            hT_new = sbig.tile([128, N], F32, tag="hT")
            nc.vector.tensor_tensor(hT_new, y, paff, OP.add)
            if l < L - 1:
                nc.vector.tensor_scalar_max(hT_new, hT_new, 0.0)
            hT = hT_new

        # ---------------- output: transpose back ----------------
        for c in range(NCHUNK):
            po = pmisc.tile([128, 128], F32, tag="pbig")
            nc.tensor.transpose(po, hT[:, bass.ts(c, 128)], ident)
            osb = shd.tile([128, 128], F32, tag="osb")
            nc.scalar.activation(osb, po, AF.Copy)
            nc.sync.dma_start(out_d[bass.ts(c, 128), :], osb)

    nc.compile()
    return nc


def _get_nc():
    if "nc" not in _CACHE:
        _CACHE["nc"] = build_nc()
    return _CACHE["nc"]


def kernel(**inputs) -> np.ndarray:
    nc = _get_nc()
    shared = {k: np.ascontiguousarray(np.asarray(inputs[k], dtype=np.float32))
              for k in ("Wp", "bp", "W_heads", "a_heads", "W_out", "a_out",
                        "ln_g", "ln_b")}
    x = np.asarray(inputs["x"], dtype=np.float32)
    adj = np.asarray(inputs["adj"], dtype=np.int32)
    in_maps = [dict(x=np.ascontiguousarray(x[b]),
                    adj=np.ascontiguousarray(adj[b]), **shared)
               for b in range(B)]
    res = run_bass_kernel_spmd(nc, in_maps, core_ids=list(range(B)))
    return np.stack([res.results[b]["out"] for b in range(B)])


if __name__ == "__main__":
    rng = np.random.default_rng(0)
    inputs = dict(
        x=rng.normal(size=(B, N, DIN)).astype(np.float32),
        adj=rng.integers(0, 2, size=(B, N, N)).astype(np.int32),
        Wp=(rng.normal(size=(DIN, F)) * 0.12).astype(np.float32),
        bp=np.zeros(F, dtype=np.float32),
        W_heads=(rng.normal(size=(L, H, F, F)) * 0.08).astype(np.float32),
        a_heads=(rng.normal(size=(L, H, 2 * F)) * 0.08).astype(np.float32),
        W_out=(rng.normal(size=(L, H * F, F)) * 0.03).astype(np.float32),
        a_out=(rng.normal(size=(L, 2 * F)) * 0.08).astype(np.float32),
        ln_g=np.ones((L, F), dtype=np.float32),
        ln_b=np.zeros((L, F), dtype=np.float32),
    )
    out = kernel(**inputs)
    print("out", out.shape, out.dtype, np.abs(out).max())
